# revision 12
# baseline (speedup 1.0000x reference)
"""Causal self-attention (B=2, T=2048, C=1024, H=16, D=64) on 8 trn2 cores.

Sharding: tensor-parallel over (batch, head-group). Core c handles batch
c//4 and heads 4*(c%4) .. 4*(c%4)+4. Each core computes its 4 heads'
QKV projection, causal attention, and the partial output projection
(W_proj row-shard). The 4 partials per batch are summed on the host
(equivalent to the Megatron all-reduce, done at gather time).

v2: software-pipelined emission tuned to keep the PE array continuously
busy (its clock p-state ramps with sustained use):
  - X^T comes pre-transposed from the host: no PE transposes, no
    PSUM->SBUF staging copies.
  - Per round r (q-chunk of 512), attention S->exp->PV chains are
    emitted with a 2-group S lookahead, and QKV projections for round
    r+1 plus output projections for earlier rounds are interleaved as
    PE filler so the PE never waits on the ACT engine's exp.
  - ACT does exp only; all bias adds / masks / normalization on DVE;
    V' ones columns are memset once (gpsimd) instead of per-round DMA.
  - yt output in fp16 (halves output DMA); host accumulates in fp32.

On-core dataflow is feature-major throughout:
  Q^T,K^T = W^T X^T ; V natural = (X^T chunk)^T Wv
  S^T[k,q] = K Q^T per 128-k-chunk (causal: q >= k-chunk start)
  P^T = exp(S^T/8) (ACT), diag-block masked (DVE)
  O'^T[128,q] += V'[k,:]^T P^T  where V' carries 64 ones cols per head,
  so rows 64:128 accumulate the softmax denominator.
  O^T = O'^T[0:64] * recip(rows 64:128).
  Y^T = W_proj^T O^T + b_proj  -> [1024, 2048] partial per core.
"""
import os
import sys
from collections import deque

import numpy as np

B, T, C = 2, 2048, 1024
H, D = 16, 64
HPC = 4                 # heads per core
QC = HPC * D            # 256 qkv cols per core
NCORES = 8
NKC = 8                 # contraction chunks over C
NT4 = T // 512          # 4 rounds of 512 q-positions
SCALE = 1.0 / np.sqrt(D)

_cache = {}


def _ensure_env():
    for p in ("/opt/trn_rl_repo", "/root/.axon_site/_ro/trn_rl_repo"):
        if os.path.isdir(p) and p not in sys.path:
            sys.path.append(p)
    jp = os.environ.get("JAX_PLATFORMS")
    if jp and "axon" not in jp and "jax" not in sys.modules:
        os.environ["JAX_PLATFORMS"] = ""


def _build_groups(r):
    """Pack the S^T chunks of round r into [128,1024] PSUM tiles.
    Returns list of groups; each group is a list of (kc, lo, n, off)."""
    lo0, hi0 = r * 512, (r + 1) * 512
    groups, cur, pos = [], [], 0
    for kc in range(4 * r + 4):
        lo = max(lo0, kc * 128)
        n = hi0 - lo
        npos = pos if pos % 512 + n <= 512 else (pos + 511) // 512 * 512
        if npos + n > 1024:
            groups.append(cur)
            cur, npos = [], 0
        cur.append((kc, lo, n, npos))
        pos = npos + n
    groups.append(cur)
    return groups


def _build():
    import concourse.bass as bass
    import concourse.bacc as bacc
    import concourse.mybir as mybir
    import concourse.tile as tile

    F32 = mybir.dt.float32
    F16 = mybir.dt.float16
    AF = mybir.ActivationFunctionType

    nc = bacc.Bacc()
    xt_d = nc.dram_tensor("xt", [C, T], F16, kind="ExternalInput")
    wq_d = nc.dram_tensor("wq", [C, QC], F16, kind="ExternalInput")
    wk_d = nc.dram_tensor("wk", [C, QC], F16, kind="ExternalInput")
    wv_d = nc.dram_tensor("wv", [C, QC], F16, kind="ExternalInput")
    bq_d = nc.dram_tensor("bq", [128, 2], F32, kind="ExternalInput")
    bk_d = nc.dram_tensor("bk", [128, 2], F32, kind="ExternalInput")
    bv_d = nc.dram_tensor("bv", [1, QC], F32, kind="ExternalInput")
    wp_d = nc.dram_tensor("wp", [QC, C], F16, kind="ExternalInput")
    bp_d = nc.dram_tensor("bp", [128, 8], F32, kind="ExternalInput")
    mask_d = nc.dram_tensor("mask", [128, 128], F16, kind="ExternalInput")
    yt_d = nc.dram_tensor("yt", [C, T], F16, kind="ExternalOutput")

    with tile.TileContext(nc) as tc:
        with tc.tile_pool(name="cst", bufs=1) as cst, \
             tc.tile_pool(name="wgt", bufs=1) as wgt, \
             tc.tile_pool(name="xin", bufs=1) as xin, \
             tc.tile_pool(name="qk", bufs=1) as qkp, \
             tc.tile_pool(name="vv", bufs=1) as vvp, \
             tc.tile_pool(name="pp", bufs=6) as ppp, \
             tc.tile_pool(name="dn", bufs=3) as dnp, \
             tc.tile_pool(name="yy", bufs=4) as yyp, \
             tc.tile_pool(name="mm", bufs=2, space="PSUM") as mmp, \
             tc.tile_pool(name="ss", bufs=2, space="PSUM") as ssp, \
             tc.tile_pool(name="po", bufs=2, space="PSUM") as pop:

            # ---- DMA issues (ordered for earliest first use) ----
            # Weight lo-halves + xt round 0 on the sync queue, hi-halves
            # and everything else on the scalar queue, interleaved so the
            # prologue's Q,K,V consumption order is fed just-in-time.
            def wtile_pair(name):
                lo = wgt.tile([128, 4 * QC], F16, tag=name + "l", name=name + "l")
                hi = wgt.tile([128, 4 * QC], F16, tag=name + "h", name=name + "h")
                return lo, hi

            def wdma(eng, wd, ws, k):
                eng.dma_start(
                    out=ws[:].rearrange("p (c n) -> p c n", n=QC),
                    in_=wd.ap()[k * 512:(k + 1) * 512, :]
                        .rearrange("(c p) n -> p c n", p=128))

            wq_lo, wq_hi = wtile_pair("wq")
            wk_lo, wk_hi = wtile_pair("wk")
            wv_lo, wv_hi = wtile_pair("wv")

            xt_t = [[xin.tile([128, 512], F16, tag=f"x{c}_{r}",
                              name=f"x{c}_{r}") for r in range(NT4)]
                    for c in range(NKC)]

            def xt_dma(eng, r, cs):
                for c in cs:
                    eng.dma_start(
                        out=xt_t[c][r][:],
                        in_=xt_d[c * 128:(c + 1) * 128,
                                 r * 512:(r + 1) * 512])

            # sync queue: wq_lo, xt r0 c0-3, wk_lo, xt r0 c4-7, wv_lo,
            # then xt rounds 1-3
            wdma(nc.sync, wq_d, wq_lo, 0)
            xt_dma(nc.sync, 0, range(4))
            wdma(nc.sync, wk_d, wk_lo, 0)
            xt_dma(nc.sync, 0, range(4, NKC))
            wdma(nc.sync, wv_d, wv_lo, 0)
            for r in range(1, NT4):
                xt_dma(nc.sync, r, range(NKC))

            # scalar queue: hi-halves, then consts, then wp
            mask = cst.tile([128, 128], F16, tag="mask")
            bq_s = cst.tile([128, 2], F32, tag="bq")
            bk_s = cst.tile([128, 2], F32, tag="bk")
            bp_s = cst.tile([128, 8], F32, tag="bp")
            bv_row = cst.tile([1, QC], F32, tag="bvr")
            wdma(nc.scalar, wq_d, wq_hi, 1)
            wdma(nc.scalar, wk_d, wk_hi, 1)
            nc.scalar.dma_start(out=bq_s[:], in_=bq_d[:])
            nc.scalar.dma_start(out=bk_s[:], in_=bk_d[:])
            wdma(nc.scalar, wv_d, wv_hi, 1)
            nc.scalar.dma_start(out=mask[:], in_=mask_d[:])
            nc.scalar.dma_start(out=bv_row[:], in_=bv_d[:])
            nc.scalar.dma_start(out=bp_s[:], in_=bp_d[:])
            wp_s = [wgt.tile([128, C], F16, tag=f"wp{k}", name=f"wp{k}")
                    for k in range(2)]
            for k in range(2):
                nc.scalar.dma_start(out=wp_s[k][:],
                                    in_=wp_d[k * 128:(k + 1) * 128, :])

            # V' ones columns memset on DVE (idle at start); bv broadcast
            # on gpsimd (once, not per round)
            bv_b = cst.tile([128, QC], F32, tag="bvb")
            nc.gpsimd.partition_broadcast(bv_b[:], bv_row[:])
            vp_s = [vvp.tile([128, HPC * 2 * D], F16, tag=f"vp{i}",
                             name=f"vp{i}") for i in range(4 * NT4)]
            for i in range(4 * NT4):
                vt3 = vp_s[i][:].rearrange("p (h e) -> p h e", e=2 * D)
                nc.vector.memset(vt3[:, :, D:2 * D], 1.0)

            # ---- persistent activations ----
            qt_s = [qkp.tile([128, T], F16, tag=f"qt{m}", name=f"qt{m}")
                    for m in range(2)]
            kt_s = [qkp.tile([128, T], F16, tag=f"kt{m}", name=f"kt{m}")
                    for m in range(2)]
            ot_s = [qkp.tile([128, T], F16, tag=f"ot{m}", name=f"ot{m}")
                    for m in range(2)]

            # ---- filler units (cycles, emit) ----
            def qk_unit(lo_w, hi_w, bs, dst, m, r):
                def emit():
                    pq = mmp.tile([128, 512], F32, tag="mm")
                    for c in range(NKC):
                        ws = lo_w if c < 4 else hi_w
                        nc.tensor.matmul(
                            pq[:],
                            ws[:, (c % 4) * QC + m * 128:
                               (c % 4) * QC + (m + 1) * 128],
                            xt_t[c][r][:],
                            start=(c == 0), stop=(c == NKC - 1))
                    nc.scalar.activation(
                        dst[m][:, r * 512:(r + 1) * 512], pq[:],
                        AF.Identity, bias=bs[:, m:m + 1], scale=1.0)
                return (4096, emit)

            def v_unit(i, r):
                def emit():
                    kc = 4 * r + i
                    pv = mmp.tile([128, 512], F32, tag="mm")
                    for c in range(NKC):
                        ws = wv_lo if c < 4 else wv_hi
                        nc.tensor.matmul(
                            pv[:, 0:QC],
                            xt_t[c][r][:, i * 128:(i + 1) * 128],
                            ws[:, (c % 4) * QC:(c % 4 + 1) * QC],
                            start=(c == 0), stop=(c == NKC - 1))
                    vt3 = vp_s[kc][:].rearrange("p (h e) -> p h e", e=2 * D)
                    nc.vector.tensor_tensor(
                        vt3[:, :, 0:D],
                        pv[:, 0:QC].rearrange("p (h d) -> p h d", d=D),
                        bv_b[:].rearrange("p (h d) -> p h d", d=D),
                        op=bass.mybir.AluOpType.add)
                return (2048, emit)

            def qkv_units(r):
                return [qk_unit(wq_lo, wq_hi, bq_s, qt_s, 0, r),
                        qk_unit(wk_lo, wk_hi, bk_s, kt_s, 0, r),
                        v_unit(0, r), v_unit(1, r),
                        qk_unit(wq_lo, wq_hi, bq_s, qt_s, 1, r),
                        qk_unit(wk_lo, wk_hi, bk_s, kt_s, 1, r),
                        v_unit(2, r), v_unit(3, r)]

            def proj_unit(r, m):
                # bias engine: ACT only where exp is not pacing (rounds
                # 0-2 filler windows have ACT slack; round-3 windows are
                # exp-paced, and at the tail ACT is idle again)
                act_bias = (r == 0) or (r == 3 and m % 2 == 0)
                # yt DMA: sync HWDGE queue (fast); the final round is
                # split across sync+gpsimd so both queues drain in
                # parallel at the tail
                dma_eng = nc.gpsimd if (r == 3 and m % 2 == 1) else nc.sync

                def emit():
                    py = mmp.tile([128, 512], F32, tag="mm")
                    for k in range(2):
                        nc.tensor.matmul(
                            py[:], wp_s[k][:, m * 128:(m + 1) * 128],
                            ot_s[k][:, r * 512:(r + 1) * 512],
                            start=(k == 0), stop=(k == 1))
                    yt_stage = yyp.tile([128, 512], F16, tag="yt")
                    if act_bias:
                        nc.scalar.activation(yt_stage[:], py[:], AF.Identity,
                                             bias=bp_s[:, m:m + 1], scale=1.0)
                    else:
                        nc.vector.tensor_scalar_add(yt_stage[:], py[:],
                                                    bp_s[:, m:m + 1])
                    dma_eng.dma_start(
                        out=yt_d[m * 128:(m + 1) * 128,
                                 r * 512:(r + 1) * 512],
                        in_=yt_stage[:])
                return (1024, emit)

            def proj_units(r):
                return [proj_unit(r, m) for m in range(8)]

            # ---- attention round with S lookahead + filler pump ----
            def attention_round(r, filler):
                lo0, hi0 = r * 512, (r + 1) * 512
                last_kc = 4 * r + 3
                groups = _build_groups(r)
                ng = len(groups)
                seq = [(h, gi) for h in range(HPC) for gi in range(ng)]
                total_att = sum(2 * sum(n for (_, _, n, _) in g)
                                for g in groups) * HPC
                total_fill = sum(cyc for cyc, _ in filler)
                fq = deque(filler)
                sp_tiles = {}
                op_tiles = {}
                emitted = 0

                def emit_S(idx):
                    h, gi = seq[idx]
                    if gi == 0:
                        op_tiles[h] = pop.tile([128, 512], F32, tag="po",
                                               name="op_tl")
                    qt_h = qt_s[h // 2][(h % 2) * 64:(h % 2) * 64 + 64, :]
                    kt_h = kt_s[h // 2][(h % 2) * 64:(h % 2) * 64 + 64, :]
                    sp = ssp.tile([128, 1024], F32, tag="ss", name="sp")
                    sp_tiles[(h, gi)] = sp
                    for (kc, lo, n, off) in groups[gi]:
                        nc.tensor.matmul(sp[:, off:off + n],
                                         kt_h[:, kc * 128:kc * 128 + 128],
                                         qt_h[:, lo:hi0],
                                         start=True, stop=True)

                att_acc = 0
                fill_acc = 0
                for ci, (h, gi) in enumerate(seq):
                    while emitted < len(seq) and emitted <= ci + 2:
                        emit_S(emitted)
                        emitted += 1
                    grp = groups[gi]
                    sp = sp_tiles.pop((h, gi))
                    end = grp[-1][3] + grp[-1][2]
                    pt = ppp.tile([128, 1024], F16, tag="p")
                    nc.scalar.activation(pt[:, 0:end], sp[:, 0:end],
                                         AF.Exp, scale=float(SCALE))
                    op_tl = op_tiles[h]
                    for (kc, lo, n, off) in grp:
                        if kc * 128 >= lo0:  # diagonal block
                            nc.vector.tensor_mul(
                                pt[:, off:off + 128],
                                pt[:, off:off + 128], mask[:])
                        nc.tensor.matmul(
                            op_tl[:, lo - lo0:512],
                            vp_s[kc][:, h * 2 * D:(h + 1) * 2 * D],
                            pt[:, off:off + n],
                            start=(kc == 0), stop=(kc == last_kc))
                    if gi == ng - 1:
                        # normalize head h
                        rc_in = dnp.tile([64, 512], F32, tag="rci")
                        nc.vector.tensor_copy(rc_in[:], op_tl[D:2 * D, :])
                        rc = dnp.tile([64, 512], F32, tag="rc")
                        nc.vector.reciprocal_approx_fast(rc[:], rc_in[:])
                        nc.vector.tensor_mul(
                            ot_s[h // 2][(h % 2) * 64:(h % 2) * 64 + 64,
                                         lo0:hi0],
                            op_tl[0:D, :], rc[:])
                    att_acc += 2 * sum(n for (_, _, n, _) in grp)
                    while fq and fill_acc * total_att < att_acc * total_fill:
                        cyc, em = fq.popleft()
                        em()
                        fill_acc += cyc
                while fq:
                    cyc, em = fq.popleft()
                    em()

            # ---- schedule ----
            # prologue: round-0 Q,K first (weights arrive first), V after
            u0 = qkv_units(0)
            for cyc, em in [u0[0], u0[1], u0[4], u0[5],
                            u0[2], u0[3], u0[6], u0[7]]:
                em()
            # round r runs attention(r) with filler:
            #   r0: QKV(1); r1: QKV(2); r2: QKV(3)+proj(0); r3: proj(1,2)
            attention_round(0, qkv_units(1))
            attention_round(1, qkv_units(2))
            attention_round(2, qkv_units(3) + proj_units(0))
            attention_round(3, proj_units(1) + proj_units(2))
            for cyc, em in proj_units(3):
                em()

    nc.finalize()
    return nc


def _get_program():
    if "nc" not in _cache:
        _ensure_env()
        _cache["nc"] = _build()
    return _cache["nc"]


def kernel(x, w_attn, b_attn, w_proj, b_proj):
    x = np.ascontiguousarray(np.asarray(x, dtype=np.float32))
    w_attn = np.ascontiguousarray(np.asarray(w_attn, dtype=np.float32))
    b_attn = np.ascontiguousarray(np.asarray(b_attn, dtype=np.float32))
    w_proj = np.ascontiguousarray(np.asarray(w_proj, dtype=np.float32))
    b_proj = np.ascontiguousarray(np.asarray(b_proj, dtype=np.float32))

    nc = _get_program()
    from concourse.bass_utils import run_bass_kernel_spmd

    mask = np.triu(np.ones((128, 128), dtype=np.float16))
    zeros_bp = np.zeros((128, 8), dtype=np.float32)
    bp_full = np.ascontiguousarray(
        b_proj.reshape(8, 128).T.astype(np.float32))
    xt_b = [np.ascontiguousarray(x[b].astype(np.float16).T)
            for b in range(B)]

    in_maps = []
    for c in range(NCORES):
        b = c // 4
        hg = c % 4
        q0 = hg * QC
        in_maps.append({
            "xt": xt_b[b],
            "wq": np.ascontiguousarray(
                w_attn[:, q0:q0 + QC].astype(np.float16)),
            "wk": np.ascontiguousarray(
                w_attn[:, C + q0:C + q0 + QC].astype(np.float16)),
            "wv": np.ascontiguousarray(
                w_attn[:, 2 * C + q0:2 * C + q0 + QC].astype(np.float16)),
            "bq": np.ascontiguousarray(
                b_attn[q0:q0 + QC].reshape(2, 128).T),
            "bk": np.ascontiguousarray(
                b_attn[C + q0:C + q0 + QC].reshape(2, 128).T),
            "bv": np.ascontiguousarray(
                b_attn[2 * C + q0:2 * C + q0 + QC].reshape(1, QC)),
            "wp": np.ascontiguousarray(
                w_proj[q0:q0 + QC, :].astype(np.float16)),
            "bp": bp_full if hg == 0 else zeros_bp,
            "mask": mask,
        })

    trace = bool(os.environ.get("KERNEL_TRACE"))
    res = run_bass_kernel_spmd(nc, in_maps, list(range(NCORES)), trace=trace)
    _cache["last_results"] = res

    out = np.empty((B, T, C), dtype=np.float32)
    for b in range(B):
        acc = res.results[4 * b]["yt"].astype(np.float32)
        for c in range(4 * b + 1, 4 * b + 4):
            acc = acc + res.results[c]["yt"].astype(np.float32)
        out[b] = acc.T
    return out
